# revision 37
# baseline (speedup 1.0000x reference)
"""Trainium2 Bass kernel for nn_Attention_47605417509124 (sparse_attention).

Reference computation (B=4, N=4096, C=256), per batch b:
    g_x     = x @ g_w.T + g_b
    theta_x = x @ theta_w.T + theta_b
    phi_x   = x @ phi_w.T + phi_b
    f       = phi_x @ theta_x.T / N          # no softmax
    y       = f @ g_x
    out     = y @ W_w.T + W_b + x

Sharding: 8 cores = 4 batches x 2 sequence halves. Each core computes the
full theta/g projections for its batch (redundantly with its pair core) and
the phi rows / score rows / output rows for its own half of the sequence.

Host-side (free) prep:
  - x[b].T passed rotated so each core's own rows sit at columns 0..2047
    (exact: stage C sums over all j, so a consistent j-permutation of
    theta/g cancels).
  - all tensors pre-arranged in SBUF-native [128, ...] layouts so every DMA
    is contiguous; weights transposed; 1/N folded into g; W_b folded into
    the residual.

All matmuls run in float32r (TF32-like reduced-precision fp32, full PE rate)
with fp32 PSUM accumulation.
"""

import numpy as np

import concourse.bass as bass
import concourse.mybir as mybir
import concourse.tile as tile
from concourse import bacc
from concourse.bass_utils import run_bass_kernel_spmd

B, N, C = 4, 4096, 256
NCORES = 8
HALF = N // 2  # sequence rows handled per core
P = 128
JT = N // P          # 32 j tiles
IT = HALF // P       # 16 i tiles

F32 = mybir.dt.float32
F32R = mybir.dt.float32r
AF = mybir.ActivationFunctionType

_CACHE = {}


def _build_module():
    nc = bacc.Bacc("TRN2", target_bir_lowering=False, debug=False,
                   num_devices=NCORES)

    # ---- external I/O (per-core shapes, SBUF-native layouts) ----
    xT_d = nc.dram_tensor("xT", [P, 2, N], F32R, kind="ExternalInput")
    thW_d = nc.dram_tensor("thW", [P, 2, C], F32R, kind="ExternalInput")
    phW_d = nc.dram_tensor("phW", [P, 2, C], F32R, kind="ExternalInput")
    gW_d = nc.dram_tensor("gW", [P, 2, C], F32R, kind="ExternalInput")
    WW_d = nc.dram_tensor("WW", [P, 2, C], F32R, kind="ExternalInput")
    thb_d = nc.dram_tensor("thb", [P, 2], F32, kind="ExternalInput")
    phb_d = nc.dram_tensor("phb", [P, 2], F32, kind="ExternalInput")
    gbb_d = nc.dram_tensor("gbb", [1, 2, C], F32, kind="ExternalInput")
    resid_d = nc.dram_tensor("resid", [P, IT, C], F32, kind="ExternalInput")
    out_d = nc.dram_tensor("out", [P, IT, C], F32, kind="ExternalOutput")

    with tile.TileContext(nc) as tc:
        with tc.tile_pool(name="big", bufs=1) as big, \
             tc.tile_pool(name="fT", bufs=4) as fTp, \
             tc.tile_pool(name="ps_work", bufs=4, space="PSUM") as psw, \
             tc.tile_pool(name="ps_acc", bufs=4, space="PSUM") as psa:

            # ---- SBUF residents ----
            xT_sb = big.tile([P, 2, N], F32R)       # rotated x[b].T  32KB/part
            thW_sb = big.tile([P, 2, C], F32R)
            phW_sb = big.tile([P, 2, C], F32R)
            gW_sb = big.tile([P, 2, C], F32R)
            WW_sb = big.tile([P, 2, C], F32R)
            thb_sb = big.tile([P, 2], F32)
            phb_sb = big.tile([P, 2], F32)
            gbb_sb = big.tile([P, 2, C], F32)
            thetaT_sb = big.tile([P, 2, N], F32R)   # theta_x.T       32KB/part
            phiT_sb = big.tile([P, 2, HALF], F32R)  # phi_x.T         16KB/part
            gx_sb = big.tile([P, JT, C], F32R)      # g_x natural     32KB/part
            yT_sb = big.tile([P, 2, HALF], F32R)    # y.T             16KB/part
            resid_sb = big.tile([P, IT, C], F32)    # also output staging

            # ---- input DMAs (order = consumption order) ----
            # All DMA triggers go via the SP/sync sequencer: triggers block
            # the issuing engine's stream while the HWDGE ring is busy, so
            # they must not come from ACT/DVE which do real copy work.
            def ld(i, dst, src):
                nc.sync.dma_start(out=dst, in_=src)

            xT_ap = xT_d.ap()
            # first matmul (ch=0) depends only on the ch=0 halves
            ld(0, phW_sb[:, 0, :], phW_d.ap()[:, 0, :])
            ld(1, xT_sb[:, 0, 0:256], xT_ap[:, 0, 0:256])
            ld(0, phW_sb[:, 1, :], phW_d.ap()[:, 1, :])
            ld(1, xT_sb[:, 1, 0:256], xT_ap[:, 1, 0:256])
            ld(0, thW_sb, thW_d.ap())
            ld(1, xT_sb[:, :, 256:512], xT_ap[:, :, 256:512])
            ld(0, phb_sb, phb_d.ap())
            ld(1, thb_sb, thb_d.ap())
            ld(0, gbb_sb, gbb_d.ap().to_broadcast([P, 2, C]))
            ld(1, gW_sb, gW_d.ap())
            ld(0, xT_sb[:, :, 512:1024], xT_ap[:, :, 512:1024])
            for q in range(2, 8):
                ld(q, xT_sb[:, :, q * 512:(q + 1) * 512],
                   xT_ap[:, :, q * 512:(q + 1) * 512])
            ld(0, WW_sb, WW_d.ap())
            ld(1, resid_sb[:, :8, :], resid_d.ap()[:, :8, :])
            ld(0, resid_sb[:, 8:, :], resid_d.ap()[:, 8:, :])

            # ---- stage A producers (phi/theta/g per 512-column chunk) ----
            def prod_phi(kc):
                subs = ([slice(0, 256), slice(256, 512)] if kc == 0
                        else [slice(kc * 512, (kc + 1) * 512)])
                for js in subs:
                    for dh in range(2):
                        ps = psw.tile([P, 512], F32, tag="work",
                                      name=f"psph{kc}")
                        w = js.stop - js.start
                        for ch in range(2):
                            nc.tensor.matmul(
                                ps[:, :w],
                                phW_sb[:, ch, dh * P:(dh + 1) * P],
                                xT_sb[:, ch, js],
                                start=(ch == 0), stop=(ch == 1))
                        nc.scalar.activation(
                            out=phiT_sb[:, dh, js], in_=ps[:, :w],
                            func=AF.Identity,
                            bias=phb_sb[:, dh:dh + 1], scale=1.0)

            def prod_theta(kc):
                subs = ([slice(0, 256), slice(256, 512)] if kc == 0
                        else [slice(kc * 512, (kc + 1) * 512)])
                for js in subs:
                    for dh in range(2):
                        ps = psw.tile([P, 512], F32, tag="work",
                                      name=f"psth{kc}")
                        w = js.stop - js.start
                        for ch in range(2):
                            nc.tensor.matmul(
                                ps[:, :w],
                                thW_sb[:, ch, dh * P:(dh + 1) * P],
                                xT_sb[:, ch, js],
                                start=(ch == 0), stop=(ch == 1))
                        nc.scalar.activation(
                            out=thetaT_sb[:, dh, js], in_=ps[:, :w],
                            func=AF.Identity,
                            bias=thb_sb[:, dh:dh + 1], scale=1.0)

            def prod_g(kc, allow_psa):
                # g_x for the chunk's 4 j tiles; two j tiles per PSUM bank.
                # psa may only be used while the psC accumulators are free.
                for q2 in range(2):
                    jp = kc * 2 + q2
                    pool, tg = ((psa, "acc") if allow_psa and jp % 2
                                else (psw, "work"))
                    ps = pool.tile([P, 512], F32, tag=tg, name=f"psg{jp}")
                    for q in range(2):
                        jt = jp * 2 + q
                        for ch in range(2):
                            nc.tensor.matmul(
                                ps[:, q * C:(q + 1) * C],
                                xT_sb[:, ch, jt * P:(jt + 1) * P],
                                gW_sb[:, ch, :],
                                start=(ch == 0), stop=(ch == 1))
                    nc.vector.tensor_add(
                        out=gx_sb[:, jp * 2:jp * 2 + 2, :],
                        in0=ps.rearrange("p (t d) -> p t d", d=C),
                        in1=gbb_sb)

            # stage A proper: only what B(ih=0, jt=0..3) needs up front --
            # phi chunks 0-1 (i cols 0:1024) and theta/g chunk 0. All other
            # chunks are produced inside the first B/C loop, where the PE
            # has slack while the rest of xT streams in.
            prod_phi(0)
            prod_theta(0)
            prod_g(0, allow_psa=True)
            prod_phi(1)

            # ---- stage D helper: out[i, e] = yT.T @ WW + resid, per pair --
            def stage_d(ihh, itp, dma=True):
                it0 = ihh * 8 + itp * 2
                ps = psw.tile([P, 512], F32, tag="work",
                              name=f"psD{ihh}_{itp}")
                for q in range(2):
                    it = it0 + q
                    for dp in range(2):
                        nc.tensor.matmul(
                            ps[:, q * C:(q + 1) * C],
                            yT_sb[:, dp, it * P:(it + 1) * P],
                            WW_sb[:, dp, :],
                            start=(dp == 0), stop=(dp == 1))
                nc.vector.tensor_add(
                    out=resid_sb[:, it0:it0 + 2, :],
                    in0=ps.rearrange("p (t d) -> p t d", d=C),
                    in1=resid_sb[:, it0:it0 + 2, :])
                if dma:
                    nc.sync.dma_start(out=out_d.ap()[:, it0:it0 + 2, :],
                                      in_=resid_sb[:, it0:it0 + 2, :])

            # ---- stages B+C, software-pipelined over j tiles ----
            # B: fT[j, i] = sum_d thetaT[d, j] * phiT[d, i]
            # C: yT[d', i] += sum_j gx[j, d'] * fT[j, i]
            for ih in range(2):
                psC = [psa.tile([P, 512], F32, tag="acc", name=f"psC{q}")
                       for q in range(4)]
                fTs = {}

                def stage_b(jt, ih=ih, fTs=fTs):
                    fT = fTp.tile([P, 2, 512], F32R, tag="fT", name=f"fT{jt}")
                    fTs[jt] = fT
                    for ck in range(2):
                        ps = psw.tile([P, 512], F32, tag="work",
                                      name=f"psB{jt}_{ck}")
                        isl = slice(ih * 1024 + ck * 512,
                                    ih * 1024 + (ck + 1) * 512)
                        for dh in range(2):
                            nc.tensor.matmul(
                                ps,
                                thetaT_sb[:, dh, jt * P:(jt + 1) * P],
                                phiT_sb[:, dh, isl],
                                start=(dh == 0), stop=(dh == 1))
                        if ck == 0:
                            nc.vector.tensor_copy(out=fT[:, ck, :], in_=ps)
                        else:
                            nc.scalar.copy(out=fT[:, ck, :], in_=ps)

                def stage_c(jt, psC=psC, fTs=fTs):
                    fT = fTs.pop(jt)
                    for dp in range(2):
                        for ck in range(2):
                            nc.tensor.matmul(
                                psC[dp * 2 + ck],
                                gx_sb[:, jt, dp * P:(dp + 1) * P],
                                fT[:, ck, :],
                                start=(jt == 0), stop=(jt == JT - 1))

                SKEW = 2
                for jt in range(SKEW):
                    stage_b(jt)
                for jt in range(SKEW, JT):
                    if ih == 0 and SKEW <= jt <= 7 + SKEW - 1:
                        kc = jt - SKEW + 1
                        if kc <= 7:
                            prod_theta(kc)
                            prod_g(kc, allow_psa=False)
                            if kc in (2, 3):
                                prod_phi(kc)
                    if ih == 1 and jt in (2, 4, 6, 8):
                        stage_d(0, jt // 2 - 1)  # spread D(ih=0) into ih=1
                    stage_b(jt)
                    stage_c(jt - SKEW)
                for jt in range(JT - SKEW, JT):
                    stage_c(jt)

                for ck in range(2):
                    for dp in range(2):
                        isl = slice(ih * 1024 + ck * 512,
                                    ih * 1024 + (ck + 1) * 512)
                        if dp == 0:
                            nc.vector.tensor_copy(out=yT_sb[:, dp, isl],
                                                  in_=psC[dp * 2 + ck])
                        else:
                            nc.scalar.copy(out=yT_sb[:, dp, isl],
                                           in_=psC[dp * 2 + ck])

                # ---- stage D for ih=1 (ih=0's is spread into this loop);
                # adds in pairs, tail DMAs grouped 4/2/1/1 so the final
                # add + out DMA chain is short
                if ih == 1:
                    for itp in range(3):
                        stage_d(1, itp, dma=False)
                        if itp == 1:
                            nc.sync.dma_start(out=out_d.ap()[:, 8:12, :],
                                              in_=resid_sb[:, 8:12, :])
                        elif itp == 2:
                            nc.sync.dma_start(out=out_d.ap()[:, 12:14, :],
                                              in_=resid_sb[:, 12:14, :])
                    for it in (14, 15):
                        ps = psw.tile([P, 512], F32, tag="work",
                                      name=f"psDs{it}")
                        for dp in range(2):
                            nc.tensor.matmul(
                                ps[:, :C],
                                yT_sb[:, dp, it * P:(it + 1) * P],
                                WW_sb[:, dp, :],
                                start=(dp == 0), stop=(dp == 1))
                        nc.vector.tensor_add(
                            out=resid_sb[:, it:it + 1, :],
                            in0=ps[:, :C].rearrange("p (t d) -> p t d", d=C),
                            in1=resid_sb[:, it:it + 1, :])
                        nc.sync.dma_start(
                            out=out_d.ap()[:, it:it + 1, :],
                            in_=resid_sb[:, it:it + 1, :])

    nc.finalize()
    return nc


def _get_module():
    if "nc" not in _CACHE:
        _CACHE["nc"] = _build_module()
    return _CACHE["nc"]


def _to_sbuf_layout(a):
    """[(o*128+p), F...] -> [128, o, F...] contiguous."""
    o = a.shape[0] // P
    return np.ascontiguousarray(
        a.reshape(o, P, *a.shape[1:]).swapaxes(0, 1))


def _prep_in_maps(x, g_w, g_b, theta_w, theta_b, phi_w, phi_b, W_w, W_b):
    x = np.ascontiguousarray(np.asarray(x, dtype=np.float32))
    f32 = np.float32

    def col2(v):  # [256] -> [128, 2] (column h = channels h*128..h*128+127)
        return np.ascontiguousarray(np.asarray(v, f32).reshape(2, P).T)

    thW = _to_sbuf_layout(np.ascontiguousarray(np.asarray(theta_w, f32).T))
    phW = _to_sbuf_layout(np.ascontiguousarray(np.asarray(phi_w, f32).T))
    gW = _to_sbuf_layout(np.ascontiguousarray(np.asarray(g_w, f32).T / N))
    WW = _to_sbuf_layout(np.ascontiguousarray(np.asarray(W_w, f32).T))
    thb = col2(theta_b)
    phb = col2(phi_b)
    gbb = np.ascontiguousarray(
        np.broadcast_to(np.asarray(g_b, f32) / N, (1, 2, C)))
    W_b = np.asarray(W_b, f32)

    in_maps = []
    for core in range(NCORES):
        b, h = core // 2, core % 2
        rows = slice(h * HALF, (h + 1) * HALF)
        other = slice(0, HALF) if h else slice(HALF, N)
        xb = x[b]
        xrot_T = np.concatenate([xb[rows], xb[other]], axis=0).T  # [C, N]
        resid = xb[rows] + W_b                                     # [HALF, C]
        in_maps.append({
            "xT": _to_sbuf_layout(np.ascontiguousarray(xrot_T)),
            "thW": thW, "phW": phW, "gW": gW, "WW": WW,
            "thb": thb, "phb": phb, "gbb": gbb,
            "resid": _to_sbuf_layout(resid),
        })
    return in_maps


def _get_runner():
    """Build the jitted 8-core executable once; reuse across kernel() calls.

    Mirrors bass2jax.run_bass_via_pjrt's multi-core branch but caches the
    jitted shard_map so repeat calls skip retracing/recompiling.
    """
    if "runner" in _CACHE:
        return _CACHE["runner"]
    import jax
    from jax.sharding import Mesh, PartitionSpec
    try:
        from jax.experimental.shard_map import shard_map
    except Exception:
        from jax.shard_map import shard_map  # newer jax
    from concourse import bass2jax, mybir as mb

    nc = _get_module()
    bass2jax.install_neuronx_cc_hook()
    partition_name = (nc.partition_id_tensor.name
                      if nc.partition_id_tensor else None)

    in_names, out_names, out_avals, zero_shapes = [], [], [], []
    for alloc in nc.m.functions[0].allocations:
        if not isinstance(alloc, mb.MemoryLocationSet):
            continue
        name = alloc.memorylocations[0].name
        if alloc.kind == "ExternalInput":
            if name != partition_name:
                in_names.append(name)
        elif alloc.kind == "ExternalOutput":
            shape = tuple(alloc.tensor_shape)
            dtype = mb.dt.np(alloc.dtype)
            out_names.append(name)
            out_avals.append(jax.core.ShapedArray(shape, dtype))
            zero_shapes.append((shape, dtype))
    n_params = len(in_names)
    all_names = in_names + out_names
    if partition_name is not None:
        all_names.append(partition_name)
    donate = tuple(range(n_params, n_params + len(out_names)))

    def _body(*args):
        operands = list(args)
        if partition_name is not None:
            operands.append(bass2jax.partition_id_tensor())
        outs = bass2jax._bass_exec_p.bind(
            *operands,
            out_avals=tuple(out_avals),
            in_names=tuple(all_names),
            out_names=tuple(out_names),
            lowering_input_output_aliases=(),
            sim_require_finite=True,
            sim_require_nnan=True,
            nc=nc,
        )
        return tuple(outs)

    try:
        devices = jax.devices("axon")[:NCORES]
    except Exception:
        devices = jax.devices()[:NCORES]
    mesh = Mesh(np.asarray(devices), ("core",))
    nin = n_params + len(out_names)
    sharded = jax.jit(
        shard_map(_body, mesh=mesh,
                  in_specs=(PartitionSpec("core"),) * nin,
                  out_specs=(PartitionSpec("core"),) * len(out_names),
                  check_rep=False),
        donate_argnums=donate, keep_unused=True)

    def run(in_maps):
        concat_in = [
            np.concatenate([np.asarray(in_maps[c][nm]) for c in range(NCORES)],
                           axis=0)
            for nm in in_names]
        concat_zeros = [np.zeros((NCORES * s[0], *s[1:]), dt)
                        for s, dt in zero_shapes]
        out_arrs = sharded(*concat_in, *concat_zeros)
        return [
            {nm: np.asarray(out_arrs[i]).reshape(NCORES, *zero_shapes[i][0])[c]
             for i, nm in enumerate(out_names)}
            for c in range(NCORES)]

    _CACHE["runner"] = run
    return run


def kernel(x, g_w, g_b, theta_w, theta_b, phi_w, phi_b, W_w, W_b):
    in_maps = _prep_in_maps(x, g_w, g_b, theta_w, theta_b, phi_w, phi_b,
                            W_w, W_b)
    try:
        results = _get_runner()(in_maps)
    except Exception:
        _CACHE.pop("runner", None)
        nc = _get_module()
        results = run_bass_kernel_spmd(
            nc, in_maps, core_ids=list(range(NCORES))).results
    out = np.empty((B, N, C), dtype=np.float32)
    for core in range(NCORES):
        b, h = core // 2, core % 2
        o = results[core]["out"]  # [128, 16, 256] = [p, t, d]
        out[b, h * HALF:(h + 1) * HALF, :] = (
            o.swapaxes(0, 1).reshape(HALF, C))
    return out


# revision 49
# speedup vs baseline: 1.0407x; 1.0407x over previous
"""Trainium2 Bass kernel for nn_Attention_47605417509124 (sparse_attention).

Reference computation (B=4, N=4096, C=256), per batch b:
    g_x     = x @ g_w.T + g_b
    theta_x = x @ theta_w.T + theta_b
    phi_x   = x @ phi_w.T + phi_b
    f       = phi_x @ theta_x.T / N          # no softmax
    y       = f @ g_x
    out     = y @ W_w.T + W_b + x

Sharding: 8 cores = 4 batches x 2 sequence halves. Each core computes the
full theta/g projections for its batch (redundantly with its pair core) and
the phi rows / score rows / output rows for its own half of the sequence.

Host-side (free) prep:
  - x[b].T passed rotated so each core's own rows sit at columns 0..2047
    (exact: stage C sums over all j, so a consistent j-permutation of
    theta/g cancels).
  - all tensors pre-arranged in SBUF-native [128, ...] layouts so every DMA
    is contiguous; weights transposed; 1/N folded into g; W_b folded into
    the residual.

All matmuls run in float32r (TF32-like reduced-precision fp32, full PE rate)
with fp32 PSUM accumulation.
"""

import numpy as np

import concourse.bass as bass
import concourse.mybir as mybir
import concourse.tile as tile
from concourse import bacc
from concourse.bass_utils import run_bass_kernel_spmd

B, N, C = 4, 4096, 256
NCORES = 8
HALF = N // 2  # sequence rows handled per core
P = 128
JT = N // P          # 32 j tiles
IT = HALF // P       # 16 i tiles

F32 = mybir.dt.float32
F32R = mybir.dt.float32r
AF = mybir.ActivationFunctionType

_CACHE = {}


def _build_module():
    nc = bacc.Bacc("TRN2", target_bir_lowering=False, debug=False,
                   num_devices=NCORES)

    # ---- external I/O (per-core shapes, SBUF-native layouts) ----
    # comb packs phW|thW|gW|WW|thb|phb along the free dim: one DMA trigger
    # (HWDGE descriptor-gen is ~625ns each, serialized, and gates the start)
    CMB = 3 * C + 2
    xT_d = nc.dram_tensor("xT", [P, 2, N], F32R, kind="ExternalInput")
    comb_d = nc.dram_tensor("comb", [P, 2, CMB], F32R, kind="ExternalInput")
    gbb_d = nc.dram_tensor("gbb", [1, 2, C], F32, kind="ExternalInput")
    resid_d = nc.dram_tensor("resid", [P, IT, C], F32, kind="ExternalInput")
    out_d = nc.dram_tensor("out", [P, IT, C], F32, kind="ExternalOutput")

    with tile.TileContext(nc) as tc:
        with tc.tile_pool(name="big", bufs=1) as big, \
             tc.tile_pool(name="fT", bufs=4) as fTp, \
             tc.tile_pool(name="ps_work", bufs=4, space="PSUM") as psw, \
             tc.tile_pool(name="ps_acc", bufs=4, space="PSUM") as psa:

            # ---- SBUF residents ----
            xT_sb = big.tile([P, 2, N], F32R)       # rotated x[b].T  32KB/part
            comb_sb = big.tile([P, 2, CMB], F32R)   # weights + biases
            phW_sb = comb_sb[:, :, 0 * C:1 * C]
            thW_sb = comb_sb[:, :, 1 * C:2 * C]
            gW_sb = comb_sb[:, :, 2 * C:3 * C]  # holds (g_w.T/N) @ W_w.T

            def thb_col(dh):
                return comb_sb[:, dh, 3 * C:3 * C + 1].bitcast(F32)

            def phb_col(dh):
                return comb_sb[:, dh, 3 * C + 1:3 * C + 2].bitcast(F32)

            gbb_sb = big.tile([P, 2, C], F32)
            thetaT_sb = big.tile([P, 2, N], F32R)   # theta_x.T       32KB/part
            phiT_sb = big.tile([P, 2, HALF], F32R)  # phi_x.T         16KB/part
            gx_sb = big.tile([P, JT, C], F32R)      # g_x natural     32KB/part
            resid_sb = big.tile([P, IT, C], F32)    # also output staging

            # ---- input DMAs (order = consumption order) ----
            # All DMA triggers go via the SP/sync sequencer: triggers block
            # the issuing engine's stream while the HWDGE ring is busy, so
            # they must not come from ACT/DVE which do real copy work.
            def ld(i, dst, src):
                nc.sync.dma_start(out=dst, in_=src)

            xT_ap = xT_d.ap()
            ld(0, comb_sb, comb_d.ap())
            ld(1, xT_sb[:, :, 0:256], xT_ap[:, :, 0:256])
            ld(0, xT_sb[:, :, 256:512], xT_ap[:, :, 256:512])
            ld(1, gbb_sb, gbb_d.ap().to_broadcast([P, 2, C]))
            ld(0, xT_sb[:, :, 512:1024], xT_ap[:, :, 512:1024])
            for q in range(2, 8):
                ld(q, xT_sb[:, :, q * 512:(q + 1) * 512],
                   xT_ap[:, :, q * 512:(q + 1) * 512])
            ld(1, resid_sb[:, :8, :], resid_d.ap()[:, :8, :])
            ld(0, resid_sb[:, 8:, :], resid_d.ap()[:, 8:, :])

            # ---- PE warm-up during the initial DMA wait ----
            # ~16 matmuls on a zeroed tile keep the PE busy from t~0 so the
            # HAM clock gate is at full rate when real work arrives; results
            # are consumed by one dummy copy and discarded.
            warm_sb = big.tile([P, 512], F32R)
            warm_dst = big.tile([P, 512], F32)
            nc.gpsimd.memset(warm_sb.bitcast(F32), 0.0)
            ps_warm = psw.tile([P, 512], F32, tag="work", name="ps_warm")
            NWARM = 16
            for wi in range(NWARM):
                nc.tensor.matmul(ps_warm, warm_sb[:, :P], warm_sb,
                                 start=(wi == 0), stop=(wi == NWARM - 1))
            nc.vector.tensor_copy(out=warm_dst, in_=ps_warm)

            # ---- stage A producers (phi/theta/g per 512-column chunk) ----
            def prod_phi(kc):
                subs = ([slice(0, 256), slice(256, 512)] if kc == 0
                        else [slice(kc * 512, (kc + 1) * 512)])
                for js in subs:
                    for dh in range(2):
                        ps = psw.tile([P, 512], F32, tag="work",
                                      name=f"psph{kc}")
                        w = js.stop - js.start
                        for ch in range(2):
                            nc.tensor.matmul(
                                ps[:, :w],
                                phW_sb[:, ch, dh * P:(dh + 1) * P],
                                xT_sb[:, ch, js],
                                start=(ch == 0), stop=(ch == 1))
                        nc.scalar.activation(
                            out=phiT_sb[:, dh, js], in_=ps[:, :w],
                            func=AF.Identity,
                            bias=phb_col(dh), scale=1.0)

            def prod_theta(kc):
                subs = ([slice(0, 256), slice(256, 512)] if kc == 0
                        else [slice(kc * 512, (kc + 1) * 512)])
                for js in subs:
                    for dh in range(2):
                        ps = psw.tile([P, 512], F32, tag="work",
                                      name=f"psth{kc}")
                        w = js.stop - js.start
                        for ch in range(2):
                            nc.tensor.matmul(
                                ps[:, :w],
                                thW_sb[:, ch, dh * P:(dh + 1) * P],
                                xT_sb[:, ch, js],
                                start=(ch == 0), stop=(ch == 1))
                        nc.scalar.activation(
                            out=thetaT_sb[:, dh, js], in_=ps[:, :w],
                            func=AF.Identity,
                            bias=thb_col(dh), scale=1.0)

            def prod_g(kc, allow_psa):
                # g_x for the chunk's 4 j tiles; two j tiles per PSUM bank.
                # psa may only be used while the psC accumulators are free.
                for q2 in range(2):
                    jp = kc * 2 + q2
                    pool, tg = ((psa, "acc") if allow_psa and jp % 2
                                else (psw, "work"))
                    ps = pool.tile([P, 512], F32, tag=tg, name=f"psg{jp}")
                    for q in range(2):
                        jt = jp * 2 + q
                        for ch in range(2):
                            nc.tensor.matmul(
                                ps[:, q * C:(q + 1) * C],
                                xT_sb[:, ch, jt * P:(jt + 1) * P],
                                gW_sb[:, ch, :],
                                start=(ch == 0), stop=(ch == 1))
                    nc.vector.tensor_add(
                        out=gx_sb[:, jp * 2:jp * 2 + 2, :],
                        in0=ps.rearrange("p (t d) -> p t d", d=C),
                        in1=gbb_sb)

            # stage A proper: only what B(ih=0, jt=0..3) needs up front --
            # phi chunks 0-1 (i cols 0:1024) and theta/g chunk 0. All other
            # chunks are produced inside the first B/C loop, where the PE
            # has slack while the rest of xT streams in.
            prod_phi(0)
            prod_theta(0)
            prod_g(0, allow_psa=True)
            prod_phi(1)

            # ---- stage D helper: out[i, e] = yT.T @ WW + resid, per pair --
            def stage_d(ihh, itp, dma=True):
                it0 = ihh * 8 + itp * 2
                ps = psw.tile([P, 512], F32, tag="work",
                              name=f"psD{ihh}_{itp}")
                for q in range(2):
                    it = it0 + q
                    for dp in range(2):
                        nc.tensor.matmul(
                            ps[:, q * C:(q + 1) * C],
                            yT_sb[:, dp, it * P:(it + 1) * P],
                            WW_sb[:, dp, :],
                            start=(dp == 0), stop=(dp == 1))
                nc.vector.tensor_add(
                    out=resid_sb[:, it0:it0 + 2, :],
                    in0=ps.rearrange("p (t d) -> p t d", d=C),
                    in1=resid_sb[:, it0:it0 + 2, :])
                if dma:
                    nc.sync.dma_start(out=out_d.ap()[:, it0:it0 + 2, :],
                                      in_=resid_sb[:, it0:it0 + 2, :])

            # ---- stages B+C, software-pipelined over j tiles ----
            # B: fT[j, i] = sum_d thetaT[d, j] * phiT[d, i]
            # C: yT[d', i] += sum_j gx[j, d'] * fT[j, i]
            for ih in range(2):
                psC = [psa.tile([P, 512], F32, tag="acc", name=f"psC{q}")
                       for q in range(4)]
                fTs = {}

                def stage_b(jt, ih=ih, fTs=fTs):
                    fT = fTp.tile([P, 2, 512], F32R, tag="fT", name=f"fT{jt}")
                    fTs[jt] = fT
                    for ck in range(2):
                        ps = psw.tile([P, 512], F32, tag="work",
                                      name=f"psB{jt}_{ck}")
                        isl = slice(ih * 1024 + ck * 512,
                                    ih * 1024 + (ck + 1) * 512)
                        for dh in range(2):
                            nc.tensor.matmul(
                                ps,
                                thetaT_sb[:, dh, jt * P:(jt + 1) * P],
                                phiT_sb[:, dh, isl],
                                start=(dh == 0), stop=(dh == 1))
                        if ck == 0:
                            nc.vector.tensor_copy(out=fT[:, ck, :], in_=ps)
                        else:
                            nc.scalar.copy(out=fT[:, ck, :], in_=ps)

                def stage_c(jt, psC=psC, fTs=fTs):
                    fT = fTs.pop(jt)
                    for dp in range(2):
                        for ck in range(2):
                            nc.tensor.matmul(
                                psC[dp * 2 + ck],
                                gx_sb[:, jt, dp * P:(dp + 1) * P],
                                fT[:, ck, :],
                                start=(jt == 0), stop=(jt == JT - 1))

                SKEW = 2
                for jt in range(SKEW):
                    stage_b(jt)
                for jt in range(SKEW, JT):
                    if ih == 0 and SKEW <= jt <= 7 + SKEW - 1:
                        kc = jt - SKEW + 1
                        if kc <= 7:
                            prod_theta(kc)
                            prod_g(kc, allow_psa=False)
                            if kc in (2, 3):
                                prod_phi(kc)
                    if ih == 1 and jt in (2, 4, 6, 8):
                        stage_d(0, jt // 2 - 1)  # spread D(ih=0) into ih=1
                    stage_b(jt)
                    stage_c(jt - SKEW)
                for jt in range(JT - SKEW, JT):
                    stage_c(jt)

                for ck in range(2):
                    for dp in range(2):
                        isl = slice(ih * 1024 + ck * 512,
                                    ih * 1024 + (ck + 1) * 512)
                        if dp == 0:
                            nc.vector.tensor_copy(out=yT_sb[:, dp, isl],
                                                  in_=psC[dp * 2 + ck])
                        else:
                            nc.scalar.copy(out=yT_sb[:, dp, isl],
                                           in_=psC[dp * 2 + ck])

                # ---- stage D for ih=1 (ih=0's is spread into this loop);
                # adds in pairs, tail DMAs grouped 4/2/1/1 so the final
                # add + out DMA chain is short
                if ih == 1:
                    for itp in range(3):
                        stage_d(1, itp, dma=False)
                        if itp == 1:
                            nc.sync.dma_start(out=out_d.ap()[:, 8:12, :],
                                              in_=resid_sb[:, 8:12, :])
                        elif itp == 2:
                            nc.sync.dma_start(out=out_d.ap()[:, 12:14, :],
                                              in_=resid_sb[:, 12:14, :])
                    for it in (14, 15):
                        ps = psw.tile([P, 512], F32, tag="work",
                                      name=f"psDs{it}")
                        for dp in range(2):
                            nc.tensor.matmul(
                                ps[:, :C],
                                yT_sb[:, dp, it * P:(it + 1) * P],
                                WW_sb[:, dp, :],
                                start=(dp == 0), stop=(dp == 1))
                        nc.vector.tensor_add(
                            out=resid_sb[:, it:it + 1, :],
                            in0=ps[:, :C].rearrange("p (t d) -> p t d", d=C),
                            in1=resid_sb[:, it:it + 1, :])
                        nc.sync.dma_start(
                            out=out_d.ap()[:, it:it + 1, :],
                            in_=resid_sb[:, it:it + 1, :])

    nc.finalize()
    return nc


def _get_module():
    if "nc" not in _CACHE:
        _CACHE["nc"] = _build_module()
    return _CACHE["nc"]


def _to_sbuf_layout(a):
    """[(o*128+p), F...] -> [128, o, F...] contiguous."""
    o = a.shape[0] // P
    return np.ascontiguousarray(
        a.reshape(o, P, *a.shape[1:]).swapaxes(0, 1))


def _prep_in_maps(x, g_w, g_b, theta_w, theta_b, phi_w, phi_b, W_w, W_b):
    x = np.ascontiguousarray(np.asarray(x, dtype=np.float32))
    f32 = np.float32

    def col2(v):  # [256] -> [128, 2] (column h = channels h*128..h*128+127)
        return np.ascontiguousarray(np.asarray(v, f32).reshape(2, P).T)

    thW = _to_sbuf_layout(np.ascontiguousarray(np.asarray(theta_w, f32).T))
    phW = _to_sbuf_layout(np.ascontiguousarray(np.asarray(phi_w, f32).T))
    # fold W into g (in float64 on host): y@W.T = f@(x@(gW.T@WW.T) + gb@WW.T)
    G2 = (np.asarray(g_w, np.float64).T / N) @ np.asarray(W_w, np.float64).T
    b2 = (np.asarray(g_b, np.float64) / N) @ np.asarray(W_w, np.float64).T
    gW = _to_sbuf_layout(np.ascontiguousarray(G2.astype(f32)))
    thb = col2(theta_b)[:, :, None]  # [128, 2, 1]
    phb = col2(phi_b)[:, :, None]
    comb = np.ascontiguousarray(
        np.concatenate([phW, thW, gW, thb, phb], axis=2))
    gbb = np.ascontiguousarray(
        np.broadcast_to(b2.astype(f32), (1, 2, C)))
    W_b = np.asarray(W_b, f32)

    in_maps = []
    for core in range(NCORES):
        b, h = core // 2, core % 2
        rows = slice(h * HALF, (h + 1) * HALF)
        other = slice(0, HALF) if h else slice(HALF, N)
        xb = x[b]
        xrot_T = np.concatenate([xb[rows], xb[other]], axis=0).T  # [C, N]
        resid = xb[rows] + W_b                                     # [HALF, C]
        in_maps.append({
            "xT": _to_sbuf_layout(np.ascontiguousarray(xrot_T)),
            "comb": comb, "gbb": gbb,
            "resid": _to_sbuf_layout(resid),
        })
    return in_maps


def _get_runner():
    """Build the jitted 8-core executable once; reuse across kernel() calls.

    Mirrors bass2jax.run_bass_via_pjrt's multi-core branch but caches the
    jitted shard_map so repeat calls skip retracing/recompiling.
    """
    if "runner" in _CACHE:
        return _CACHE["runner"]
    import jax
    from jax.sharding import Mesh, PartitionSpec
    try:
        from jax.experimental.shard_map import shard_map
    except Exception:
        from jax.shard_map import shard_map  # newer jax
    from concourse import bass2jax, mybir as mb

    nc = _get_module()
    bass2jax.install_neuronx_cc_hook()
    partition_name = (nc.partition_id_tensor.name
                      if nc.partition_id_tensor else None)

    in_names, out_names, out_avals, zero_shapes = [], [], [], []
    for alloc in nc.m.functions[0].allocations:
        if not isinstance(alloc, mb.MemoryLocationSet):
            continue
        name = alloc.memorylocations[0].name
        if alloc.kind == "ExternalInput":
            if name != partition_name:
                in_names.append(name)
        elif alloc.kind == "ExternalOutput":
            shape = tuple(alloc.tensor_shape)
            dtype = mb.dt.np(alloc.dtype)
            out_names.append(name)
            out_avals.append(jax.core.ShapedArray(shape, dtype))
            zero_shapes.append((shape, dtype))
    n_params = len(in_names)
    all_names = in_names + out_names
    if partition_name is not None:
        all_names.append(partition_name)
    donate = tuple(range(n_params, n_params + len(out_names)))

    def _body(*args):
        operands = list(args)
        if partition_name is not None:
            operands.append(bass2jax.partition_id_tensor())
        outs = bass2jax._bass_exec_p.bind(
            *operands,
            out_avals=tuple(out_avals),
            in_names=tuple(all_names),
            out_names=tuple(out_names),
            lowering_input_output_aliases=(),
            sim_require_finite=True,
            sim_require_nnan=True,
            nc=nc,
        )
        return tuple(outs)

    try:
        devices = jax.devices("axon")[:NCORES]
    except Exception:
        devices = jax.devices()[:NCORES]
    mesh = Mesh(np.asarray(devices), ("core",))
    nin = n_params + len(out_names)
    sharded = jax.jit(
        shard_map(_body, mesh=mesh,
                  in_specs=(PartitionSpec("core"),) * nin,
                  out_specs=(PartitionSpec("core"),) * len(out_names),
                  check_rep=False),
        donate_argnums=donate, keep_unused=True)

    def run(in_maps):
        concat_in = [
            np.concatenate([np.asarray(in_maps[c][nm]) for c in range(NCORES)],
                           axis=0)
            for nm in in_names]
        concat_zeros = [np.zeros((NCORES * s[0], *s[1:]), dt)
                        for s, dt in zero_shapes]
        out_arrs = sharded(*concat_in, *concat_zeros)
        return [
            {nm: np.asarray(out_arrs[i]).reshape(NCORES, *zero_shapes[i][0])[c]
             for i, nm in enumerate(out_names)}
            for c in range(NCORES)]

    _CACHE["runner"] = run
    return run


def kernel(x, g_w, g_b, theta_w, theta_b, phi_w, phi_b, W_w, W_b):
    in_maps = _prep_in_maps(x, g_w, g_b, theta_w, theta_b, phi_w, phi_b,
                            W_w, W_b)
    try:
        results = _get_runner()(in_maps)
    except Exception:
        _CACHE.pop("runner", None)
        nc = _get_module()
        results = run_bass_kernel_spmd(
            nc, in_maps, core_ids=list(range(NCORES))).results
    out = np.empty((B, N, C), dtype=np.float32)
    for core in range(NCORES):
        b, h = core // 2, core % 2
        o = results[core]["out"]  # [128, 16, 256] = [p, t, d]
        out[b, h * HALF:(h + 1) * HALF, :] = (
            o.swapaxes(0, 1).reshape(HALF, C))
    return out
